# revision 13
# baseline (speedup 1.0000x reference)
"""GATv2 (2-layer, PyG GATv2Conv semantics) on 8 Trainium2 NeuronCores.

Edge-parallel via dst-sort; one static SPMD Bass/Tile program for all
cores (all graph irregularity is carried in input data):

  host:   sort edges by dst; split nodes into 8 contiguous ranges with
          ~E/8 edges each; group each core's nodes into 128-node blocks;
          pad each block's edge list to S_FIX subblocks of 128 edges;
          pre-gather x[src], x[dst] per edge (bf16, feature-major).
  L1:     m_pre = Wl1.x_src + We1.ef + Wr1.x_dst   (PE, feat-major)
          m  = leaky_relu(m_pre)                   (ACT Lrelu)
          logitsT per subblock (lhsT=m_sub, rhs=att_blk), ex = exp
          xl per subblock (lhsT=x_srcT_sub, rhs=Wl1) -> edge-major
          msgs = xl * ex(head-bcast)               (DVE)
          onehot[e,n] = (dst_rel == iota)          (DVE is_equal)
          agg[n,:] += onehot^T @ [msgs | ex]       (PE, per subblock)
          per block: h = elu(agg/den + b1); xl2 = h@Wl2, xr2 = h@Wr2.
  comm:   AllGather xl2 -> shared table [8*NK_PAD, 16].
  L2:     indirect-DMA gather xl2[src], xr2[dst]; e2 = ef@We2;
          single-head attention, same one-hot aggregation;
          h2 = agg/den + b2; log_softmax per row.

The program is JIT-specialized to the observed graph structure (node
ranges, S_FIX); compiled once per structure and cached in-process.
"""

import numpy as np
import ml_dtypes

P = 128
DIN = 64
F1 = 128          # HEADS * DH
H = 8
DH = 16
DOUT = 16

BF16 = ml_dtypes.bfloat16

N_CORES = 8

# ---------------------------------------------------------------------------
# Host-side planning / sharding
# ---------------------------------------------------------------------------


def _plan(edge_index, n_nodes, n_cores):
    dst = np.asarray(edge_index[1], dtype=np.int64)
    E = dst.shape[0]
    order = np.argsort(dst, kind="stable")
    deg = np.bincount(dst, minlength=n_nodes)
    cumdeg = np.cumsum(deg)

    bounds = [0]
    for k in range(1, n_cores):
        target = E * k // n_cores
        b = int(np.searchsorted(cumdeg, target))
        bounds.append(min(max(b, bounds[-1] + 1), n_nodes - (n_cores - k)))
    bounds.append(n_nodes)

    nk_max = max(bounds[k + 1] - bounds[k] for k in range(n_cores))
    NB = (nk_max + P - 1) // P

    max_sub = 1
    for k in range(n_cores):
        lo, hi = bounds[k], bounds[k + 1]
        for b in range(NB):
            nlo = lo + b * P
            nhi = min(lo + (b + 1) * P, hi)
            if nlo >= hi:
                continue
            bd = int(cumdeg[nhi - 1] - (cumdeg[nlo - 1] if nlo > 0 else 0))
            max_sub = max(max_sub, (bd + P - 1) // P)
    # chunklets are 6 subblocks (L1) / 12 subblocks (L2); S_FIX % 6 == 0
    # and NB*S_FIX % 12 == 0.
    S_FIX = ((max_sub + 5) // 6) * 6
    if (NB * S_FIX) % 12 != 0:
        S_FIX += 6
    return dict(bounds=bounds, NB=NB, S_FIX=S_FIX, order=order, cumdeg=cumdeg)


def _shard(inputs, plan, n_cores):
    x = np.asarray(inputs["x"], np.float32)
    ef = np.asarray(inputs["edge_feats"], np.float32)
    ei = np.asarray(inputs["edge_index"])
    src_all = np.asarray(ei[0], np.int64)
    dst_all = np.asarray(ei[1], np.int64)
    n_nodes = x.shape[0]

    bounds = plan["bounds"]
    NB, S_FIX = plan["NB"], plan["S_FIX"]
    order = plan["order"]
    cumdeg = plan["cumdeg"]
    NK_PAD = NB * P
    SLOTS = NB * S_FIX * P
    SCOLS = NB * S_FIX

    node_core = np.zeros(n_nodes, np.int64)
    node_local = np.zeros(n_nodes, np.int64)
    for k in range(n_cores):
        lo, hi = bounds[k], bounds[k + 1]
        node_core[lo:hi] = k
        node_local[lo:hi] = np.arange(hi - lo)
    node_trow = node_core * NK_PAD + node_local

    xT = np.ascontiguousarray(x.T)

    per_core = []
    for k in range(n_cores):
        lo, hi = bounds[k], bounds[k + 1]
        slot_edge = np.full(SLOTS, -1, np.int64)
        for b in range(NB):
            nlo = lo + b * P
            nhi = min(lo + (b + 1) * P, hi)
            if nlo >= hi:
                continue
            s = int(cumdeg[nlo - 1]) if nlo > 0 else 0
            e = int(cumdeg[nhi - 1])
            cnt = e - s
            assert cnt <= S_FIX * P, f"nodeblock overflow: {cnt}"
            base = b * S_FIX * P
            slot_edge[base:base + cnt] = order[s:e]

        valid = slot_edge >= 0
        se = np.clip(slot_edge, 0, None)
        esrc = np.where(valid, src_all[se], 0)
        edst = np.where(valid, dst_all[se], 0)

        xef = np.zeros((P, SLOTS), BF16)
        xs = xT[:, esrc]
        xs[:, ~valid] = 0
        xef[0:DIN] = xs.astype(BF16)
        efv = ef[se]
        efv[~valid] = 0
        xef[DIN:P] = efv.T.astype(BF16)

        xd = xT[:, edst]
        xd[:, ~valid] = 0
        xd = np.ascontiguousarray(xd.astype(BF16))

        blk_of_slot = np.arange(SLOTS) // (S_FIX * P)
        drel = np.where(valid, node_local[edst] - blk_of_slot * P, -1)
        dstrel = np.ascontiguousarray(
            drel.reshape(SCOLS, P).T.astype(np.float32)).astype(BF16)

        srcoff = np.where(valid, node_trow[esrc], 0).astype(np.int32)
        dstoff = np.where(valid, node_local[edst], 0).astype(np.int32)
        srcoff = np.ascontiguousarray(srcoff.reshape(SCOLS, P).T)
        dstoff = np.ascontiguousarray(dstoff.reshape(SCOLS, P).T)

        per_core.append(dict(arrays=dict(
            xefT=xef, xdstT=xd, dstrel=dstrel,
            srcoff=srcoff, dstoff=dstoff), lo=lo, hi=hi))

    Wl1 = np.asarray(inputs["Wl1"], np.float32)
    We1 = np.asarray(inputs["We1"], np.float32)
    att1 = np.asarray(inputs["att1"], np.float32)
    att_blk = np.zeros((F1, H), np.float32)
    for h in range(H):
        att_blk[h * DH:(h + 1) * DH, h] = att1[h]
    We2 = np.asarray(inputs["We2"], np.float32)

    consts = dict(
        Wstack=np.concatenate([Wl1, We1], 0).astype(BF16),
        Wr1=np.asarray(inputs["Wr1"], np.float32).astype(BF16),
        Wl1=Wl1.astype(BF16),
        We2rep=np.concatenate([We2, We2], 0).astype(BF16),
        att_blk=att_blk.astype(BF16),
        Wl2=np.asarray(inputs["Wl2"], np.float32).astype(BF16),
        Wr2=np.asarray(inputs["Wr2"], np.float32).astype(BF16),
        att2rep=np.tile(np.asarray(inputs["att2"], np.float32).reshape(1, DOUT),
                        (P, 1)).astype(BF16),
        b1rep=np.tile(np.asarray(inputs["b1"], np.float32).reshape(1, F1),
                      (P, 1)),
        b2rep=np.tile(np.asarray(inputs["b2"], np.float32).reshape(1, DOUT),
                      (P, 1)),
        iotaC=np.tile(np.arange(P, dtype=np.float32).reshape(1, P),
                      (P, 1)).astype(BF16),
        ident=np.eye(P, dtype=np.float32).astype(BF16),
    )
    meta = dict(NB=NB, S_FIX=S_FIX, NK_PAD=NK_PAD, SLOTS=SLOTS, SCOLS=SCOLS)
    return per_core, consts, meta


# ---------------------------------------------------------------------------
# Device program
# ---------------------------------------------------------------------------


def build_program(meta, n_cores, sim_safe=False):
    import concourse.bass as bass
    import concourse.bacc as bacc
    import concourse.mybir as mybir
    import concourse.tile as tile

    f32 = mybir.dt.float32
    bf16 = mybir.dt.bfloat16
    i32 = mybir.dt.int32
    AF = mybir.ActivationFunctionType
    OP = mybir.AluOpType
    AX = mybir.AxisListType

    NB = meta["NB"]
    S_FIX = meta["S_FIX"]
    NK_PAD = meta["NK_PAD"]
    SLOTS = meta["SLOTS"]
    SCOLS = meta["SCOLS"]
    CPB = S_FIX // 6
    T1 = 6 * P
    L2S = 12
    NSC = SCOLS // L2S
    assert SCOLS % L2S == 0

    nc = bacc.Bacc("TRN2", target_bir_lowering=False, debug=False,
                   num_devices=n_cores)

    def din(name, shape, dt):
        return nc.declare_dram_parameter(name, list(shape), dt, isOutput=False)

    xefT_d = din("xefT", (P, SLOTS), bf16)
    xdstT_d = din("xdstT", (DIN, SLOTS), bf16)
    dstrel_d = din("dstrel", (P, SCOLS), bf16)
    srcoff_d = din("srcoff", (P, SCOLS), i32)
    dstoff_d = din("dstoff", (P, SCOLS), i32)
    const_decls = dict(
        Wstack=((P, F1), bf16), Wr1=((DIN, F1), bf16), Wl1=((DIN, F1), bf16),
        We2rep=((P, DOUT), bf16), att_blk=((F1, H), bf16),
        Wl2=((F1, DOUT), bf16), Wr2=((F1, DOUT), bf16),
        att2rep=((P, DOUT), bf16), b1rep=((P, F1), f32),
        b2rep=((P, DOUT), f32), iotaC=((P, P), bf16), ident=((P, P), bf16),
    )
    const_d = {k: din(k, shp, dt) for k, (shp, dt) in const_decls.items()}

    out_h_d = nc.declare_dram_parameter("out_h", [NK_PAD, DOUT], f32,
                                        isOutput=True)
    out_ls_d = nc.declare_dram_parameter("out_ls", [NK_PAD, DOUT], f32,
                                         isOutput=True)

    xl2_local = nc.dram_tensor("xl2_local", [NK_PAD, DOUT], f32)
    xr2_local = nc.dram_tensor("xr2_local", [NK_PAD, DOUT], f32)
    xl2_all = nc.dram_tensor(
        "xl2_all", [n_cores * NK_PAD, DOUT], f32,
        addr_space="Shared" if n_cores > 4 else "Local")

    with tile.TileContext(nc) as tc:
        with (
            tc.tile_pool(name="const", bufs=1) as cpool,
            tc.tile_pool(name="sb", bufs=3) as sb,
            tc.tile_pool(name="fin", bufs=2) as fin,
            tc.tile_pool(name="big", bufs=1) as big,
        ):
            C = {}
            for name, (shp, dt) in const_decls.items():
                t = cpool.tile(list(shp), dt, tag=name)
                nc.sync.dma_start(out=t[:], in_=const_d[name][:])
                C[name] = t
            dstrel = cpool.tile([P, SCOLS], bf16, tag="dstrel")
            nc.sync.dma_start(out=dstrel[:], in_=dstrel_d[:])

            h_all = big.tile([P, NB, F1], bf16, tag="h_all")

            # =========================================================
            # Layer 1
            # =========================================================
            with (
                tc.tile_pool(name="ps_pre", bufs=2, space="PSUM") as ps_pre,
                tc.tile_pool(name="ps_xlog", bufs=1, space="PSUM") as ps_xlog,
                tc.tile_pool(name="ps_agg", bufs=2, space="PSUM") as ps_agg,
            ):
                for blk in range(NB):
                    agg = ps_agg.tile([P, F1 + H], f32, tag="agg")
                    for ck in range(CPB):
                        base = (blk * S_FIX + ck * 6) * P
                        scol = blk * S_FIX + ck * 6
                        xef = sb.tile([P, T1], bf16, tag="xef")
                        nc.sync.dma_start(out=xef[:],
                                          in_=xefT_d[:, base:base + T1])
                        xd = sb.tile([DIN, T1], bf16, tag="xd")
                        nc.sync.dma_start(out=xd[:],
                                          in_=xdstT_d[:, base:base + T1])

                        pre = ps_pre.tile([P, T1], f32, tag="pre")
                        for sl in (slice(0, 512), slice(512, T1)):
                            nc.tensor.matmul(out=pre[:, sl], lhsT=C["Wstack"][:],
                                             rhs=xef[:, sl],
                                             start=True, stop=False)
                            nc.tensor.matmul(out=pre[:, sl], lhsT=C["Wr1"][:],
                                             rhs=xd[:, sl],
                                             start=False, stop=True)
                        m_sb = sb.tile([P, T1], bf16, tag="m_sb")
                        if sim_safe:
                            # CoreSim lacks Lrelu: leaky = relu(x)-0.2*relu(-x)
                            nc.scalar.activation(m_sb[:], pre[:], AF.Relu)
                            lk = sb.tile([P, T1], bf16, tag="lk")
                            nc.scalar.activation(lk[:], pre[:], AF.Relu,
                                                 scale=-1.0)
                            nc.vector.tensor_scalar(lk[:], lk[:], 0.2, None,
                                                    OP.mult)
                            nc.vector.tensor_tensor(out=m_sb[:], in0=m_sb[:],
                                                    in1=lk[:],
                                                    op=OP.subtract)
                        else:
                            nc.scalar.activation(m_sb[:], pre[:], AF.Lrelu,
                                                 alpha=0.2)

                        xlog = ps_xlog.tile([P, 6 * F1 + 6 * H], f32,
                                            tag="xlog")
                        for s in range(6):
                            esl = slice(s * P, (s + 1) * P)
                            nc.tensor.matmul(out=xlog[:, s * F1:(s + 1) * F1],
                                             lhsT=xef[0:DIN, esl],
                                             rhs=C["Wl1"][:],
                                             start=True, stop=True)
                        for s in range(6):
                            esl = slice(s * P, (s + 1) * P)
                            nc.tensor.matmul(
                                out=xlog[:, 6 * F1 + s * H:6 * F1 + (s + 1) * H],
                                lhsT=m_sb[:, esl], rhs=C["att_blk"][:],
                                start=True, stop=True)

                        msgs = sb.tile([P, 6, F1 + H], bf16, tag="msgs")
                        exT = sb.tile([P, 6, H], bf16, tag="exT")
                        nc.scalar.activation(
                            exT[:],
                            xlog[:, 6 * F1:6 * (F1 + H)].rearrange(
                                "p (s h) -> p s h", s=6),
                            AF.Exp)
                        nc.scalar.activation(
                            msgs[:, :, F1:F1 + H],
                            xlog[:, 6 * F1:6 * (F1 + H)].rearrange(
                                "p (s h) -> p s h", s=6),
                            AF.Exp)

                        onehot = sb.tile([P, 6, P], bf16, tag="onehot")
                        nc.vector.tensor_tensor(
                            out=onehot[:],
                            in0=dstrel[:, scol:scol + 6].rearrange(
                                "p (s o) -> p s o", o=1).to_broadcast(
                                    [P, 6, P]),
                            in1=C["iotaC"][:].rearrange(
                                "p (o n) -> p o n", o=1).to_broadcast(
                                    [P, 6, P]),
                            op=OP.is_equal)

                        nc.vector.tensor_tensor(
                            out=msgs[:, :, 0:F1].rearrange(
                                "p s (h c) -> p s h c", h=H),
                            in0=xlog[:, 0:6 * F1].rearrange(
                                "p (s h c) -> p s h c", s=6, h=H),
                            in1=exT[:].rearrange(
                                "p s (h o) -> p s h o", o=1).to_broadcast(
                                    [P, 6, H, DH]),
                            op=OP.mult)

                        first = ck == 0
                        last = ck == CPB - 1
                        for s in range(6):
                            nc.tensor.matmul(out=agg[:],
                                             lhsT=onehot[:, s, :],
                                             rhs=msgs[:, s, :],
                                             start=(first and s == 0),
                                             stop=(last and s == 5))

                    # ---- finalize nodeblock --------------------------
                    rec = fin.tile([P, H], f32, tag="rec")
                    nc.vector.tensor_scalar(rec[:], agg[:, F1:F1 + H],
                                            1e-30, None, OP.add)
                    nc.vector.reciprocal(rec[:], rec[:])
                    hdiv = fin.tile([P, F1], f32, tag="hdiv")
                    nc.vector.tensor_tensor(
                        out=hdiv[:].rearrange("p (h c) -> p h c", h=H),
                        in0=agg[:, 0:F1].rearrange("p (h c) -> p h c", h=H),
                        in1=rec[:].rearrange("p (h o) -> p h o",
                                             o=1).to_broadcast([P, H, DH]),
                        op=OP.mult)
                    nc.vector.tensor_tensor(out=hdiv[:], in0=hdiv[:],
                                            in1=C["b1rep"][:], op=OP.add)
                    # elu(x) = relu(x) + exp(min(x,0)) - 1
                    tmin = fin.tile([P, F1], f32, tag="tmin")
                    nc.vector.tensor_scalar(tmin[:], hdiv[:], 0.0, None,
                                            OP.min)
                    texp = fin.tile([P, F1], f32, tag="texp")
                    nc.scalar.activation(texp[:], tmin[:], AF.Exp)
                    trel = fin.tile([P, F1], f32, tag="trel")
                    nc.scalar.activation(trel[:], hdiv[:], AF.Relu)
                    nc.vector.tensor_tensor(out=trel[:], in0=trel[:],
                                            in1=texp[:], op=OP.add)
                    nc.vector.tensor_scalar(h_all[:, blk, :], trel[:], -1.0,
                                            None, OP.add)

                    hT_ps = ps_xlog.tile([P, P], bf16, tag="xlog")
                    nc.tensor.transpose(out=hT_ps[:], in_=h_all[:, blk, :],
                                        identity=C["ident"][:])
                    hT_sb = fin.tile([P, P], bf16, tag="hT_sb")
                    nc.scalar.copy(out=hT_sb[:], in_=hT_ps[:])
                    x2_ps = ps_xlog.tile([P, 2 * DOUT], f32, tag="xlog")
                    nc.tensor.matmul(out=x2_ps[:, 0:DOUT], lhsT=hT_sb[:],
                                     rhs=C["Wl2"][:], start=True, stop=True)
                    nc.tensor.matmul(out=x2_ps[:, DOUT:2 * DOUT],
                                     lhsT=hT_sb[:], rhs=C["Wr2"][:],
                                     start=True, stop=True)
                    x2_sb = fin.tile([P, 2 * DOUT], f32, tag="x2_sb")
                    nc.vector.tensor_copy(out=x2_sb[:], in_=x2_ps[:])
                    nc.sync.dma_start(
                        out=xl2_local[blk * P:(blk + 1) * P, :],
                        in_=x2_sb[:, 0:DOUT])
                    nc.sync.dma_start(
                        out=xr2_local[blk * P:(blk + 1) * P, :],
                        in_=x2_sb[:, DOUT:2 * DOUT])

            # =========================================================
            # AllGather xl2
            # =========================================================
            nc.gpsimd.collective_compute(
                "AllGather", mybir.AluOpType.bypass,
                replica_groups=[list(range(n_cores))],
                ins=[xl2_local[:]],
                outs=[xl2_all[:]],
            )

            # =========================================================
            # Layer 2
            # =========================================================
            srcoff_sb = cpool.tile([P, SCOLS], i32, tag="srcoff")
            nc.sync.dma_start(out=srcoff_sb[:], in_=srcoff_d[:])
            dstoff_sb = cpool.tile([P, SCOLS], i32, tag="dstoff")
            nc.sync.dma_start(out=dstoff_sb[:], in_=dstoff_d[:])

            xl2g = big.tile([P, SCOLS, DOUT], bf16, tag="xl2g")
            xr2g = big.tile([P, SCOLS, DOUT], bf16, tag="xr2g")
            NGS = 6 if SCOLS % 6 == 0 else 1
            gcols = SCOLS // NGS
            for g in range(NGS):
                gs = slice(g * gcols, (g + 1) * gcols)
                nc.gpsimd.indirect_dma_start(
                    out=xl2g[:, gs, :], out_offset=None,
                    in_=xl2_all[:, :],
                    in_offset=bass.IndirectOffsetOnAxis(
                        ap=srcoff_sb[:, gs], axis=0))
                nc.gpsimd.indirect_dma_start(
                    out=xr2g[:, gs, :], out_offset=None,
                    in_=xr2_local[:, :],
                    in_offset=bass.IndirectOffsetOnAxis(
                        ap=dstoff_sb[:, gs], axis=0))

            agg2_live = {}
            with (
                tc.tile_pool(name="ps_e2", bufs=2, space="PSUM") as ps_e2,
                tc.tile_pool(name="ps_agg2", bufs=2, space="PSUM") as ps_agg2,
            ):
                for sc in range(NSC):
                    base = sc * L2S * P
                    scol = sc * L2S
                    xef = sb.tile([P, L2S * P], bf16, tag="xef2")
                    nc.sync.dma_start(out=xef[:],
                                      in_=xefT_d[:, base:base + L2S * P])
                    e2 = ps_e2.tile([P, L2S, DOUT], f32, tag="e2")
                    for s in range(L2S):
                        nc.tensor.matmul(out=e2[:, s, :],
                                         lhsT=xef[DIN:P, s * P:(s + 1) * P],
                                         rhs=C["We2rep"][DIN:P, :],
                                         start=True, stop=True)
                    m2 = sb.tile([P, L2S, DOUT], bf16, tag="m2")
                    nc.vector.tensor_tensor(out=m2[:],
                                            in0=xl2g[:, scol:scol + L2S, :],
                                            in1=xr2g[:, scol:scol + L2S, :],
                                            op=OP.add)
                    nc.vector.tensor_tensor(out=m2[:], in0=m2[:], in1=e2[:],
                                            op=OP.add)
                    m2b = sb.tile([P, L2S, DOUT], bf16, tag="m2b")
                    nc.vector.tensor_scalar(m2b[:], m2[:], 0.2, None, OP.mult)
                    nc.vector.tensor_tensor(out=m2[:], in0=m2[:], in1=m2b[:],
                                            op=OP.max)
                    t2 = sb.tile([P, L2S, DOUT], bf16, tag="t2")
                    nc.vector.tensor_tensor(
                        out=t2[:], in0=m2[:],
                        in1=C["att2rep"][:].rearrange(
                            "p (o c) -> p o c", o=1).to_broadcast(
                                [P, L2S, DOUT]),
                        op=OP.mult)
                    lg2 = sb.tile([P, L2S], f32, tag="lg2")
                    nc.vector.tensor_reduce(out=lg2[:], in_=t2[:],
                                            axis=AX.X, op=OP.add)
                    ex2 = sb.tile([P, L2S], bf16, tag="ex2")
                    nc.scalar.activation(ex2[:], lg2[:], AF.Exp)
                    msgs2 = sb.tile([P, L2S, DOUT + 1], bf16, tag="msgs2")
                    nc.vector.tensor_tensor(
                        out=msgs2[:, :, 0:DOUT],
                        in0=xl2g[:, scol:scol + L2S, :],
                        in1=ex2[:].rearrange("p (s o) -> p s o",
                                             o=1).to_broadcast(
                                                 [P, L2S, DOUT]),
                        op=OP.mult)
                    nc.vector.tensor_copy(out=msgs2[:, :, DOUT], in_=ex2[:])

                    onehot2 = sb.tile([P, L2S, P], bf16, tag="onehot2")
                    nc.vector.tensor_tensor(
                        out=onehot2[:],
                        in0=dstrel[:, scol:scol + L2S].rearrange(
                            "p (s o) -> p s o", o=1).to_broadcast(
                                [P, L2S, P]),
                        in1=C["iotaC"][:].rearrange(
                            "p (o n) -> p o n", o=1).to_broadcast(
                                [P, L2S, P]),
                        op=OP.is_equal)

                    for s in range(L2S):
                        gsub = scol + s
                        blk = gsub // S_FIX
                        sin = gsub % S_FIX
                        if sin == 0:
                            agg2_live[blk] = ps_agg2.tile([P, DOUT + 1], f32,
                                                          tag="agg2",
                                                          name="agg2")
                        a2 = agg2_live[blk]
                        nc.tensor.matmul(out=a2[:], lhsT=onehot2[:, s, :],
                                         rhs=msgs2[:, s, :],
                                         start=(sin == 0),
                                         stop=(sin == S_FIX - 1))
                        if sin != S_FIX - 1:
                            continue
                        # ---- finalize nodeblock ----------------------
                        del agg2_live[blk]
                        rec2 = fin.tile([P, 1], f32, tag="rec2")
                        nc.vector.tensor_scalar(rec2[:],
                                                a2[:, DOUT:DOUT + 1],
                                                1e-30, None, OP.add)
                        nc.vector.reciprocal(rec2[:], rec2[:])
                        h2 = fin.tile([P, DOUT], f32, tag="h2")
                        nc.vector.tensor_tensor(
                            out=h2[:], in0=a2[:, 0:DOUT],
                            in1=rec2[:].to_broadcast([P, DOUT]),
                            op=OP.mult)
                        nc.vector.tensor_tensor(out=h2[:], in0=h2[:],
                                                in1=C["b2rep"][:], op=OP.add)
                        nc.sync.dma_start(
                            out=out_h_d[blk * P:(blk + 1) * P, :],
                            in_=h2[:])
                        e_t = fin.tile([P, DOUT], f32, tag="e_t")
                        sume = fin.tile([P, 1], f32, tag="sume")
                        nc.scalar.activation(e_t[:], h2[:], AF.Exp,
                                             accum_out=sume[:])
                        lns = fin.tile([P, 1], f32, tag="lns")
                        nc.scalar.activation(lns[:], sume[:], AF.Ln)
                        ls = fin.tile([P, DOUT], f32, tag="ls")
                        nc.vector.tensor_tensor(
                            out=ls[:], in0=h2[:],
                            in1=lns[:].to_broadcast([P, DOUT]),
                            op=OP.subtract)
                        nc.sync.dma_start(
                            out=out_ls_d[blk * P:(blk + 1) * P, :],
                            in_=ls[:])

    nc.compile()
    return nc


# ---------------------------------------------------------------------------
# Entry point
# ---------------------------------------------------------------------------

_CACHE = {}

# test-harness knobs (ignored by graders that just call kernel()):
PROFILE = False          # run with NTFF tracing, stash result in LAST_RESULT
LAST_RESULT = None


def kernel(**inputs):
    n_cores = N_CORES
    ei = np.asarray(inputs["edge_index"])
    N = int(np.asarray(inputs["x"]).shape[0])

    plan = _plan(ei, N, n_cores)
    per_core, consts, meta = _shard(inputs, plan, n_cores)

    key = (N, ei.shape[1], meta["NB"], meta["S_FIX"], n_cores)
    if key not in _CACHE:
        _CACHE[key] = build_program(meta, n_cores)
    nc = _CACHE[key]

    from concourse.bass_utils import run_bass_kernel_spmd
    in_maps = []
    for k in range(n_cores):
        m = dict(per_core[k]["arrays"])
        m.update(consts)
        in_maps.append(m)

    res = run_bass_kernel_spmd(nc, in_maps, list(range(n_cores)),
                               trace=PROFILE)
    global LAST_RESULT
    LAST_RESULT = res
    out_h = np.zeros((N, DOUT), np.float32)
    out_ls = np.zeros((N, DOUT), np.float32)
    for k in range(n_cores):
        lo, hi = per_core[k]["lo"], per_core[k]["hi"]
        out_h[lo:hi] = res.results[k]["out_h"][0:hi - lo]
        out_ls[lo:hi] = res.results[k]["out_ls"][0:hi - lo]
    return out_h, out_ls
